# revision 37
# baseline (speedup 1.0000x reference)
"""Trainium2 Bass kernel for nn_DenseEquivariantIrrep.

The reference module (group-Fourier transform -> per-irrep block matmul over
input channels -> inverse transform -> bias) is linear in x, so the whole
pipeline collapses into a single fused operator W of shape (IN_F*N_SYMM,
OUT_F*N_SYMM) = (1024, 1024) plus a bias that only depends on the output
feature index.  W is tiny and depends only on the small parameter tensors, so
it is precomputed on the host in float64; the device work is a pure
data-parallel (65536, 1024) @ (1024, 1024) matmul, sharded over batch across
8 NeuronCores (8192 rows each).

Structure exploited to reach (and then lower) the HBM roofline:

1. The reference masks the kernel to the even group elements, which form an
   index-2 subgroup (D_16 in D_32).  Group convolution with a
   subgroup-supported kernel never mixes the two cosets, so under an
   even/odd permutation of the group axis W is two independent 512x512
   blocks (cross blocks numerically zero -- checked at runtime, with a
   dense-W fallback).  K halves: PE streaming and W traffic halve.

2. Everything on the wire is bf16 (tolerance is 2e-2; bf16 lands ~2e-3):
   x is cast+transposed+coset-permuted on the host, W is bf16, and y is
   written back as bf16 in a transposed parity-packed layout that the host
   unpacks.  HBM traffic per core drops from 64 MB (fp32 in/out) to 32 MB.

3. W (not x) is the stationary operand: each stationary [128k x 128n] W
   block is reused across 8 consecutive matmuls over batch columns, so the
   weight-load path (which at one LDWEIGHTS per matmul dominated the fp32r
   version) amortizes away, and the PSUM result comes out as yT which DMAs
   out in large contiguous runs with no DVE scatter.

Per-core device pipeline: x^T lives fully in SBUF (16 MB bf16, loaded in
1-2 MB DMAs overlapped with compute); for each (parity, output 128-chunk,
batch 4096-block): 4 K-chunks x 8 batch-quarters of bf16 matmuls accumulate
into 8 PSUM banks -> DVE casts each bank to bf16 into a [128, 4096] tile ->
one 1 MB DMA stores it to yT.  The host rebuilds y (b, f, 2u+parity) and
adds the bias.
"""

import sys

import numpy as np

sys.path.insert(0, "/opt/trn_rl_repo")

import ml_dtypes

import concourse.mybir as mybir
import concourse.tile as tile
from concourse import bacc
from concourse.bass_utils import run_bass_kernel_spmd

N_CORES = 8
B = 65536
IN_F = 16
OUT_F = 16
N_SYMM = 64
K = IN_F * N_SYMM   # 1024 contraction dim
N = OUT_F * N_SYMM  # 1024 output dim
P = 128
ROWS = B // N_CORES  # 8192 rows per core
KC = K // P          # 8 contraction chunks
F32 = mybir.dt.float32
F32R = mybir.dt.float32r
BF16 = mybir.dt.bfloat16
NPBF16 = np.dtype(ml_dtypes.bfloat16)


def _build_w(kernel_params, kernel_idx, fwd_mat, inv_mat):
    """Fused linear operator W[(c,g), (f,g')] in float64, cast late."""
    kp = np.asarray(kernel_params, np.float64)
    fwd = np.asarray(fwd_mat, np.float64)
    inv = np.asarray(inv_mat, np.float64)
    kern = np.zeros((OUT_F, IN_F, N_SYMM), np.float64)
    kern[:, :, np.asarray(kernel_idx)] = kp
    kf = kern @ fwd  # (f, c, m)
    # wh[(c, m'), (f, m'')]: the per-irrep block matmul in Fourier space.
    wh = np.zeros((IN_F, N_SYMM, OUT_F, N_SYMM), np.float64)
    for n in range(4):  # 1-dim irreps
        wh[:, n, :, n] = kf[:, :, n].T
    for n in range(15):  # 2-dim irreps: (i,j) x (j,k) -> (i,k)
        base = 4 + 4 * n
        for i in range(2):
            for j in range(2):
                for k_ in range(2):
                    wh[:, base + 2 * i + j, :, base + 2 * i + k_] = (
                        kf[:, :, base + 2 * j + k_].T
                    )
    t = np.tensordot(fwd, wh, axes=(1, 1))  # (g, c, f, m'')
    w4 = np.tensordot(t, inv, axes=(3, 0))  # (g, c, f, g')
    w = w4.transpose(1, 0, 2, 3).reshape(K, N)
    return np.ascontiguousarray(w)


_NC_CACHE = {}


def _build_nc_dense():
    """fp32r dense fallback (only if the parity split doesn't apply)."""
    if "dense" in _NC_CACHE:
        return _NC_CACHE["dense"]

    SB = 1024
    N_SUPER = ROWS // SB
    nc = bacc.Bacc(
        "TRN2",
        target_bir_lowering=False,
        debug=False,
        enable_asserts=False,
        num_devices=N_CORES,
    )
    xt_d = nc.dram_tensor("xt", [K, ROWS], F32R, kind="ExternalInput").ap()
    w_d = nc.dram_tensor("w", [K, N], F32R, kind="ExternalInput").ap()
    bias_d = nc.dram_tensor("biasb", [P, N], F32, kind="ExternalInput").ap()
    y_d = nc.dram_tensor("y", [ROWS, N], F32, kind="ExternalOutput").ap()

    with tile.TileContext(nc) as tc:
        with (
            tc.tile_pool(name="const", bufs=1) as cpool,
            tc.tile_pool(name="xs", bufs=2) as xpool,
            tc.tile_pool(name="ys", bufs=4) as ypool,
            tc.tile_pool(name="psy", bufs=4, space="PSUM") as psypool,
        ):
            w_sb = cpool.tile([P, KC, N], F32R, tag="w")
            for kc in range(KC):
                nc.scalar.dma_start(
                    out=w_sb[:, kc], in_=w_d[kc * P : (kc + 1) * P, :]
                )
            bias_sb = cpool.tile([P, N], F32, tag="bias")
            nc.scalar.dma_start(out=bias_sb, in_=bias_d)

            for st in range(N_SUPER):
                b0 = st * SB
                x_sb = xpool.tile([P, KC, SB], F32R, tag="x", name=f"x_{st}")
                if st == 0:
                    for kc in range(KC):
                        for h in range(SB // 512):
                            nc.sync.dma_start(
                                out=x_sb[:, kc, h * 512 : (h + 1) * 512],
                                in_=xt_d[
                                    kc * P : (kc + 1) * P,
                                    b0 + h * 512 : b0 + (h + 1) * 512,
                                ],
                            )
                else:
                    nc.sync.dma_start(
                        out=x_sb,
                        in_=xt_d[:, b0 : b0 + SB].rearrange("(a p) b -> p a b", p=P),
                    )

                for pair in range(SB // P // 2):
                    y_sb = ypool.tile([P, 2, N], F32, tag="y", name=f"y_{st}_{pair}")
                    for sub in range(2):
                        bt = pair * 2 + sub
                        ps_y = [
                            psypool.tile(
                                [P, 512], F32, tag=f"psy{nh}",
                                name=f"psy{nh}_{st}_{bt}",
                            )
                            for nh in range(2)
                        ]
                        for kc in range(KC):
                            lhsT = x_sb[:, kc, bt * P : (bt + 1) * P]
                            for nh in range(2):
                                nc.tensor.matmul(
                                    ps_y[nh],
                                    lhsT,
                                    w_sb[:, kc, nh * 512 : (nh + 1) * 512],
                                    start=(kc == 0),
                                    stop=(kc == KC - 1),
                                )
                        for nh in range(2):
                            nc.vector.tensor_add(
                                y_sb[:, sub, nh * 512 : (nh + 1) * 512],
                                ps_y[nh],
                                bias_sb[:, nh * 512 : (nh + 1) * 512],
                            )
                    nc.scalar.dma_start(
                        out=y_d[
                            b0 + pair * 2 * P : b0 + (pair + 1) * 2 * P, :
                        ].rearrange("(a p) n -> p a n", p=P),
                        in_=y_sb,
                    )

    nc.compile()
    _NC_CACHE["dense"] = nc
    return nc


def _build_nc_parity_bf16():
    """bf16 half-K kernel with W stationary and transposed packed output.

    Inputs (per core):
      xt [1024, 8192] bf16 -- x^T, rows coset-permuted: rows 0-511 = (c, t)
        for g=2t, rows 512-1023 = (c, t) for g=2t+1.
      w  [1024, 512] bf16 -- rows follow the same order; w[:512] = W_ee,
        w[512:] = W_oo, each mapping to 512 packed output columns (f, u).
    Output:
      yt [1024, 8192] bf16 -- y^T, rows = par*512 + f*32 + u, i.e. the
        natural n = f*64 + 2u + par, parity-packed.  Host unpacks.
    """
    if "parity16" in _NC_CACHE:
        return _NC_CACHE["parity16"]

    nc = bacc.Bacc(
        "TRN2",
        target_bir_lowering=False,
        debug=False,
        enable_asserts=False,
        num_devices=N_CORES,
    )
    xt_d = nc.dram_tensor("xt", [K, ROWS], BF16, kind="ExternalInput").ap()
    w_d = nc.dram_tensor("w", [K, 512], BF16, kind="ExternalInput").ap()
    yt_d = nc.dram_tensor("yt", [N, ROWS], BF16, kind="ExternalOutput").ap()

    # Batch-column blocks per parity, ramped: the first par0 block needs
    # only 512 KB of x in SBUF (so compute starts as soon as the cold DMA
    # ring delivers it), and the last par1 block keeps the end-of-kernel
    # drain (casts + final store) short.  Four nch-rounds reuse each block
    # before moving on.
    BLOCKS0 = ((0, 1024), (1024, 2048), (2048, 4096), (4096, 8192))
    BLOCKS1 = ((0, 4096), (4096, 6144), (6144, 7680), (7680, 8192))

    with tile.TileContext(nc) as tc:
        with (
            tc.tile_pool(name="const", bufs=1) as cpool,
            tc.tile_pool(name="ys", bufs=6) as ypool,
            tc.tile_pool(name="psy", bufs=8, space="PSUM") as pspool,
        ):
            w_sb = cpool.tile([P, KC, 512], BF16, tag="w")
            for kc in range(4, KC):
                # par1's W rides the scalar ring: needed only at ~65us.
                nc.scalar.dma_start(
                    out=w_sb[:, kc], in_=w_d[kc * P : (kc + 1) * P, :]
                )
            # Whole x^T shard stays resident (16 MB bf16), all on the sync
            # HWDGE ring in consumption order.  (Splitting onto SWDGE
            # measured far slower; splitting onto the scalar ring deadlocks
            # against the y stores scheduled there.)
            x_sb = cpool.tile([P, KC, ROWS], BF16, tag="x")
            # First-use interleave on the sync ring: round 1's kcl chain
            # consumes (W kc, x kc) pairs in this exact order, so the first
            # matmul is ready after 384 KB instead of waiting for the
            # late-starting scalar ring to deliver W.
            for kcg in range(4):
                nc.sync.dma_start(
                    out=w_sb[:, kcg], in_=w_d[kcg * P : (kcg + 1) * P, :]
                )
                nc.sync.dma_start(
                    out=x_sb[:, kcg, 0:1024],
                    in_=xt_d[kcg * P : (kcg + 1) * P, 0:1024],
                )
            for c0, c1 in BLOCKS0[1:]:
                for kcg in range(4):
                    nc.sync.dma_start(
                        out=x_sb[:, kcg, c0:c1],
                        in_=xt_d[kcg * P : (kcg + 1) * P, c0:c1],
                    )
            for c0, c1 in ((0, 4096), (4096, ROWS)):
                for kcg in range(4, KC):
                    nc.sync.dma_start(
                        out=x_sb[:, kcg, c0:c1],
                        in_=xt_d[kcg * P : (kcg + 1) * P, c0:c1],
                    )

            # blk outside nch: the four nch rounds of a block reuse the same
            # x columns, giving the input DMA a full block of slack.
            for par in range(2):
                for blk, (b0, b1) in enumerate(BLOCKS0 if par == 0 else BLOCKS1):
                    bw = b1 - b0
                    nq = bw // 512
                    for nch in range(4):
                        y_t = ypool.tile(
                            [P, 4096], BF16, tag="y", name=f"y_{par}_{nch}_{blk}"
                        )
                        pss = [
                            pspool.tile(
                                [P, 512], F32, tag="ps",
                                name=f"ps_{par}_{nch}_{blk}_{q}",
                            )
                            for q in range(nq)
                        ]
                        # kcl outer: one stationary W block feeds nq matmuls.
                        for kcl in range(4):
                            kc = par * 4 + kcl
                            lhsT = w_sb[:, kc, nch * P : (nch + 1) * P]
                            for q in range(nq):
                                c0 = b0 + q * 512
                                nc.tensor.matmul(
                                    pss[q],
                                    lhsT,
                                    x_sb[:, kc, c0 : c0 + 512],
                                    start=(kcl == 0),
                                    stop=(kcl == 3),
                                )
                        # Evictions stay on DVE only: ACT reads of PSUM
                        # contend with PE result writes (+50ns on every MM).
                        # 4096-wide blocks store in two halves so the first
                        # half overlaps the second half's casts.
                        for q in range(nq):
                            nc.vector.tensor_copy(
                                y_t[:, q * 512 : (q + 1) * 512], pss[q]
                            )
                        row0 = par * 512 + nch * P
                        nh = 2 if bw == 4096 else 1
                        for h in range(nh):
                            hw = bw // nh
                            nc.scalar.dma_start(
                                out=yt_d[
                                    row0 : row0 + P,
                                    b0 + h * hw : b0 + (h + 1) * hw,
                                ],
                                in_=y_t[:, h * hw : (h + 1) * hw],
                            )

    nc.compile()
    _NC_CACHE["parity16"] = nc
    return nc


_COSET_PERM = np.concatenate(
    [
        (np.arange(IN_F)[:, None] * N_SYMM + 2 * np.arange(32)[None, :]).ravel(),
        (np.arange(IN_F)[:, None] * N_SYMM + 2 * np.arange(32)[None, :] + 1).ravel(),
    ]
)


def _prepare(x, kernel_params, bias, kernel_idx, fwd_mat, inv_mat):
    w = _build_w(kernel_params, kernel_idx, fwd_mat, inv_mat)

    # Coset split: valid iff W has no even<->odd coupling on the group axis
    # (always true for the reference's even-element kernel mask).
    w4 = w.reshape(IN_F, N_SYMM, OUT_F, N_SYMM)
    ev, od = np.arange(0, N_SYMM, 2), np.arange(1, N_SYMM, 2)
    cross = max(
        np.abs(w4[:, ev][:, :, :, od]).max(),
        np.abs(w4[:, od][:, :, :, ev]).max(),
    )
    parity_ok = cross <= 1e-6 * max(np.abs(w).max(), 1e-30)

    if parity_ok:
        w_ee = w4[:, ev][:, :, :, ev].reshape(512, 512)
        w_oo = w4[:, od][:, :, :, od].reshape(512, 512)
        w_packed = np.concatenate([w_ee, w_oo], axis=0).astype(NPBF16)
        x16 = np.asarray(x, np.float32).reshape(N_CORES, ROWS, K).astype(NPBF16)
        xt_all = np.ascontiguousarray(
            x16.transpose(0, 2, 1)[:, _COSET_PERM, :]
        )
        nc = _build_nc_parity_bf16()
        in_maps = [
            {"xt": xt_all[i], "w": w_packed} for i in range(N_CORES)
        ]
        return nc, in_maps, "parity16"

    bias_flat = np.repeat(np.asarray(bias, np.float64), N_SYMM).astype(np.float32)
    bias_b = np.ascontiguousarray(np.broadcast_to(bias_flat, (P, N)))
    x_flat = np.asarray(x, np.float32).reshape(N_CORES, ROWS, K)
    xt_all = np.ascontiguousarray(x_flat.transpose(0, 2, 1))
    nc = _build_nc_dense()
    in_maps = [
        {"xt": xt_all[i], "w": w.astype(np.float32), "biasb": bias_b}
        for i in range(N_CORES)
    ]
    return nc, in_maps, "dense"


def kernel(x, kernel_params, bias, kernel_idx, fwd_mat, inv_mat):
    nc, in_maps, mode = _prepare(
        x, kernel_params, bias, kernel_idx, fwd_mat, inv_mat
    )
    res = run_bass_kernel_spmd(nc, in_maps, core_ids=list(range(N_CORES)))
    if mode == "parity16":
        yt = np.stack([res.results[i]["yt"] for i in range(N_CORES)], axis=0)
        # yt rows: par*512 + f*32 + u  ->  y[b, f, 2u+par]
        y = (
            yt.reshape(N_CORES, 2, OUT_F, 32, ROWS)
            .transpose(0, 4, 2, 3, 1)
            .reshape(B, OUT_F, N_SYMM)
            .astype(np.float32)
        )
        y += np.asarray(bias, np.float32)[None, :, None]
        return np.ascontiguousarray(y)
    y = np.concatenate([res.results[i]["y"] for i in range(N_CORES)], axis=0)
    return np.ascontiguousarray(y.reshape(B, OUT_F, N_SYMM).astype(np.float32))
